# revision 1
# baseline (speedup 1.0000x reference)
"""Causal attention pixel block kernel for Trainium2 (8 NeuronCores).

Problem: 3 directional stacks x batch 1 x 8 heads of causal attention over
S=2048 flattened spatial positions, head dim 8 (64 channels total), fp32.

Sharding: the 3*1*8 = 24 (stack, head) units are data/head-parallel; each of
the 8 cores processes 3 units end-to-end (full 2048x2048 logits for its
units). The causal mask is the deterministic lower-triangular mask from the
reference; it is implemented on-chip (block skipping + a triangular mask on
diagonal blocks), so the attn_mask input never needs to reach the device.

Per-unit device pipeline (all fp32):
  scoresT[j, i] = sum_c k[c, j] q[c, i]      (PE, K=8 matmuls, j-tiles of 128)
  wT = exp(scoresT / sqrt(8))                (ScalarE, PSUM -> SBUF)
  diagonal blocks: wT *= upper-tri mask      (VectorE)
  outT[c, i] = sum_j vaug[j, c] wT[j, i]     (PE, accumulated over j-tiles)
    where vaug has a ones-column: row c=8 of outT is the softmax denominator
  out = outT[0:8] * recip(outT[8])           (VectorE + GpSimd broadcast)

The i-axis is processed in halves of 1024 so PSUM holds two double-buffered
[128, 1024] score tiles plus two [9, 1024] output accumulators (8 banks).
"""

import math

import numpy as np

import concourse.bass as bass
import concourse.tile as tile
from concourse import bacc, mybir
from concourse.bass_utils import run_bass_kernel_spmd
from concourse.masks import make_upper_triangular

N_CORES = 8
STACK, B, C, D, H, W = 3, 1, 64, 8, 16, 16
S = D * H * W                  # 2048 attention positions
NH = 8                         # num heads
CK = C // NH                   # head dim = 8
UNITS = STACK * B * NH         # 24
UPC = UNITS // N_CORES         # 3 units per core
NJT = S // 128                 # 16 j-tiles per unit
AVW = 40                       # AV lhsT width: ones col at 0 (rowsum lands at
                               # PSUM partition 0 where the fast-reciprocal
                               # custom op can read it), v in cols 32..39
                               # (partition 32 is a legal engine base)
HALF = S // 2                  # i-axis processed in halves of 1024
SCALE = CK ** -0.5

F32 = mybir.dt.float32
# fp32 matmuls stream at 4 cycles/row on the PE; float32r (same bits) streams
# at 1 cycle/row for moving dims >= 256.
F32R = mybir.dt.float32r

# tuning knobs (module-level so sweep scripts can override before build)
QK_BUFS = 2      # PSUM double-buffering for score tiles ([128, HALF] = 2 banks)
AV_BUFS = 2      # PSUM buffering for the [40, 1024] output accumulators (2 banks)
W_BUFS = 4       # SBUF buffering for exp'd score tiles
O_BUFS = 6       # SBUF buffering for the normalize/output tiles
DIAG_LAST = False # emit the mask-dependent diagonal AV chunk after the others
PE_WARMUP = 4      # dummy matmuls to release the HAM clock throttle early
FINE_TAIL = True   # 512-wide normalize chunks on the very last half only
BCAST_DMA = False  # broadcast recip row via DRAM-bounce DMA vs gpsimd
NORM_CHUNK = 1024  # width of the normalize/output chains (512 or 1024)
ABLATE = ""        # timing ablations: "qk" | "exp" | "av" | "" (full)
REPS = 1         # repeat the whole compute (for calibration benchmarks only)


def _emit(tc: tile.TileContext, q_d, k_d, v_d, o_d):
    nc = tc.nc
    Exp = mybir.ActivationFunctionType.Exp

    with (
        tc.tile_pool(name="singles", bufs=1) as singles,
        tc.tile_pool(name="w", bufs=W_BUFS) as wpool,
        tc.tile_pool(name="out", bufs=O_BUFS) as opool,
        tc.tile_pool(name="qk", bufs=QK_BUFS, space="PSUM") as qkpool,
        tc.tile_pool(name="av", bufs=AV_BUFS, space="PSUM") as avpool,
        tc.tile_pool(name="dram", bufs=O_BUFS, space="DRAM") as dpool,
    ):
        # trigger the ACT exp table load immediately so it overlaps the
        # input DMAs instead of stalling the first real exp (~2.7us)
        warm = singles.tile([1, 1], F32)
        nc.vector.memset(warm, 0.0)
        nc.scalar.activation(warm, warm, Exp, scale=1.0)

        q_sb = singles.tile([CK, UPC, S], F32R)
        k_sb = singles.tile([CK, UPC, S], F32R)
        v_sb = singles.tile([128, UPC, NJT, AVW], F32R)
        # priority slices: just what the first QK row needs (k j-tile 0 and
        # the first half of q for unit 0), so compute starts ~2us earlier
        nc.sync.dma_start(out=k_sb[:, 0, 0:128], in_=k_d.ap()[:, 0, 0:128])
        nc.sync.dma_start(out=q_sb[:, 0, 0:HALF], in_=q_d.ap()[:, 0, 0:HALF])
        # bulk loads (exclude the priority slices to avoid a rewrite stall)
        nc.sync.dma_start(out=k_sb[:, 0, 128:S], in_=k_d.ap()[:, 0, 128:S])
        nc.sync.dma_start(out=q_sb[:, 0, HALF:S], in_=q_d.ap()[:, 0, HALF:S])
        nc.sync.dma_start(out=v_sb[:, 0, :, :], in_=v_d.ap()[:, 0, :, :])
        for u in range(1, UPC):
            nc.sync.dma_start(out=k_sb[:, u, :], in_=k_d.ap()[:, u, :])
            nc.sync.dma_start(out=q_sb[:, u, :], in_=q_d.ap()[:, u, :])
            nc.sync.dma_start(out=v_sb[:, u, :, :], in_=v_d.ap()[:, u, :, :])

        # trimask[p, f] = 1.0 if f >= p else 0.0 (keep j <= i on diag blocks)
        trimask = singles.tile([128, 128], F32)
        make_upper_triangular(nc, trimask[:], val=1.0, diag=True)

        if PE_WARMUP:
            # dummy matmuls during the input DMA wait: ~3.4us of PE activity
            # releases the HAM clock throttle (1.2 -> 2.4 GHz) before the
            # first real QK matmul
            wsrc = singles.tile([CK, 512], F32R)
            nc.vector.memset(wsrc.bitcast(F32), 0.0)
            wp = qkpool.tile([128, HALF], F32, tag='qk')
            for _ in range(PE_WARMUP):
                nc.tensor.matmul(
                    wp[:, 0:512],
                    lhsT=wsrc[:, 0:128],
                    rhs=wsrc,
                    start=True,
                    stop=True,
                )

        for _rep in range(REPS):
            for u in range(UPC):
                for hf in range(2):
                    base = hf * HALF           # absolute i offset of this half
                    jt_end = (hf + 1) * (HALF // 128)
                    av = avpool.tile([AVW, HALF], F32)

                    def emit_qk(jt):
                        s0 = max(jt * 128, base) - base
                        qk = qkpool.tile([128, HALF], F32)
                        for c0 in range(0, HALF, 512):
                            lo = max(c0, s0)
                            if lo >= c0 + 512:
                                continue
                            nc.tensor.matmul(
                                qk[:, lo:c0 + 512],
                                lhsT=k_sb[:, u, jt * 128:(jt + 1) * 128],
                                rhs=q_sb[:, u, base + lo:base + c0 + 512],
                                start=True,
                                stop=True,
                            )
                        return qk

                    qk = emit_qk(0)
                    for jt in range(jt_end):
                        s0 = max(jt * 128, base) - base
                        wt = wpool.tile([128, HALF], F32R)
                        if ABLATE != "qk":
                            nc.scalar.activation(
                                wt[:, s0:HALF], qk[:, s0:HALF], Exp, scale=SCALE
                            )
                        # issue the next row's QK before this row's AV so the
                        # PE keeps ScalarE fed instead of running in lockstep
                        if jt + 1 < jt_end:
                            qk = emit_qk(jt + 1)
                        if ABLATE in ("qk", "exp"):
                            continue
                        if jt * 128 >= base:
                            # diagonal block: zero out j > i entries
                            nc.vector.tensor_mul(
                                wt[:, s0:s0 + 128], wt[:, s0:s0 + 128], trimask
                            )
                        for c0 in range(0, HALF, 512):
                            lo = max(c0, s0)
                            if lo >= c0 + 512:
                                continue
                            last_jt = min(jt_end - 1, (base + c0 + 512) // 128 - 1)
                            nc.tensor.matmul(
                                av[:, lo:c0 + 512],
                                lhsT=v_sb[:, u, jt, :],
                                rhs=wt[:, lo:c0 + 512],
                                start=(jt == 0),
                                stop=(jt == last_jt),
                                skip_group_check=True,
                            )
                        # normalize any chunk-wide output chunk that just
                        # finished accumulating (low chunks finish early):
                        # out = outT[32:40] * recip(rowsum row 0)
                        if ABLATE:
                            continue
                        nchunk = NORM_CHUNK
                        if FINE_TAIL and u == UPC - 1 and hf == 1:
                            nchunk = 512
                        for c in range(HALF // nchunk):
                            cl, ch = nchunk * c, nchunk * (c + 1)
                            if min(jt_end - 1, (base + ch) // 128 - 1) != jt:
                                continue
                            sl = slice(cl, ch)
                            r = opool.tile([1, nchunk], F32)
                            nc.vector.reciprocal_approx_fast(
                                out=r, in_=av[0:1, sl]
                            )
                            rb = opool.tile([CK, nchunk], F32)
                            if BCAST_DMA:
                                # bounce through DRAM: a partition-stride-0
                                # read (broadcast) is only legal from DRAM,
                                # and DMA costs no compute-engine time
                                rd = dpool.tile([1, NORM_CHUNK], F32)
                                nc.sync.dma_start(out=rd, in_=r)
                                rd_b = bass.AP(
                                    tensor=rd.tensor, offset=rd.offset,
                                    ap=[[0, CK]] + list(rd.ap[1:]),
                                )
                                nc.sync.dma_start(out=rb, in_=rd_b)
                            else:
                                nc.gpsimd.partition_broadcast(rb, r, channels=CK)
                            osb = opool.tile([CK, nchunk], F32)
                            nc.vector.tensor_mul(osb, av[32:32 + CK, sl], rb)
                            nc.sync.dma_start(
                                out=o_d.ap()[u, :, base + cl:base + ch],
                                in_=osb,
                            )


_PROGRAM = None


def _get_program():
    global _PROGRAM
    if _PROGRAM is None:
        nc = bacc.Bacc(
            "TRN2",
            target_bir_lowering=False,
            debug=False,
            num_devices=N_CORES,
        )
        q_d = nc.declare_dram_parameter("q", [CK, UPC, S], F32R, isOutput=False)
        k_d = nc.declare_dram_parameter("k", [CK, UPC, S], F32R, isOutput=False)
        v_d = nc.declare_dram_parameter(
            "vaug", [128, UPC, NJT, AVW], F32R, isOutput=False
        )
        o_d = nc.declare_dram_parameter("o", [UPC, CK, S], F32, isOutput=True)
        with tile.TileContext(nc) as tc:
            _emit(tc, q_d, k_d, v_d, o_d)
        if not nc.is_finalized():
            nc.finalize()
        _PROGRAM = nc
    return _PROGRAM


# test.py can flip this on to capture an NTFF trace / exec time.
TRACE = False
LAST_RESULTS = None


def kernel(keys, queries, values, attn_mask, num_heads):
    global LAST_RESULTS
    nh = int(num_heads)
    assert nh == NH, f"compiled for num_heads={NH}, got {nh}"
    assert keys.shape == (STACK, B, C, D, H, W)

    # (stack*b, head, ck, seq)
    q = np.ascontiguousarray(queries, np.float32).reshape(STACK * B, NH, CK, S)
    k = np.ascontiguousarray(keys, np.float32).reshape(STACK * B, NH, CK, S)
    v = np.ascontiguousarray(values, np.float32).reshape(STACK * B, NH, CK, S)

    in_maps = []
    for core in range(N_CORES):
        units = range(core * UPC, (core + 1) * UPC)
        qs = np.stack([q[u // NH, u % NH] for u in units], 1)  # [CK, UPC, S]
        ks = np.stack([k[u // NH, u % NH] for u in units], 1)
        vt = np.stack([v[u // NH, u % NH] for u in units], 0)  # [UPC, CK, S]
        vaug = np.zeros((128, UPC, NJT, AVW), np.float32)
        vaug[:, :, :, 32:32 + CK] = vt.reshape(UPC, CK, NJT, 128).transpose(3, 0, 2, 1)
        vaug[:, :, :, 0] = 1.0
        in_maps.append(
            {
                "q": np.ascontiguousarray(qs),
                "k": np.ascontiguousarray(ks),
                "vaug": vaug,
            }
        )

    nc = _get_program()
    kwargs = {}
    if TRACE:
        kwargs = dict(trace=True, trace_cores=[0])
    LAST_RESULTS = run_bass_kernel_spmd(
        nc, in_maps, core_ids=list(range(N_CORES)), **kwargs
    )

    out = np.empty((STACK * B, NH, CK, S), np.float32)
    for core in range(N_CORES):
        o = LAST_RESULTS.results[core]["o"]  # [UPC, CK, S]
        for j, u in enumerate(range(core * UPC, (core + 1) * UPC)):
            out[u // NH, u % NH] = o[j]
    return out.reshape(STACK, B, C, D, H, W)

